# revision 6
# baseline (speedup 1.0000x reference)
"""Associative-embedding (AE) loss kernel for Trainium2, 8 NeuronCores.

Strategy (data-parallel over batch, per the sharding hint):
  - B=8 images, 8 cores -> one image per core.
  - Per core: gather the 30x17 tag values tags[k, idx[m,k]] with a single
    indirect (gather) DMA -- only 510 x 4B of the 17MB tag map is touched.
  - Tiny DVE/ACT/PE compute produces the per-image (pull, push) scalars.
  - AllReduce(add) across the 8 cores; every core writes the batch totals.
"""

import numpy as np

import concourse.bass as bass
import concourse.bacc as bacc
import concourse.tile as tile
from concourse import mybir
from concourse.bass_utils import run_bass_kernel_spmd

B, K, HW, M = 8, 17, 262144, 30
NCORES = 8
MP = 32  # person dim padded to the DVE stream-transpose block size

F32 = mybir.dt.float32
I32 = mybir.dt.int32
AX = mybir.AxisListType
OP = mybir.AluOpType
ACT = mybir.ActivationFunctionType


def build_nc(finalize=True):
    nc = bacc.Bacc(None, num_devices=NCORES)
    tags = nc.declare_dram_parameter("tags", [K, HW], F32, isOutput=False)
    kp = nc.declare_dram_parameter("kp", [M, K, 2], I32, isOutput=False)
    out = nc.declare_dram_parameter("out", [1, 2], F32, isOutput=True)

    with tile.TileContext(nc) as tc:
        with (
            tc.tile_pool(name="sb", bufs=1) as sb,
            tc.tile_pool(name="ps", bufs=1, space="PSUM") as ps,
            tc.tile_pool(name="dram", bufs=1, space="DRAM") as dram,
        ):
            # keypoints -> SBUF; partition = person m
            kp_t = sb.tile([M, K, 2], I32)
            nc.gpsimd.dma_start(out=kp_t[:], in_=kp[:, :, :])
            idx = kp_t[:, :, 0]
            vis = kp_t[:, :, 1]

            # flat gather index: idx[m,k] + k*HW (iota step is int16-limited,
            # so emit k then scale by HW on the DVE)
            flat = sb.tile([M, K], I32)
            nc.gpsimd.iota(flat[:], pattern=[[1, K]], base=0, channel_multiplier=0)
            nc.vector.tensor_scalar(
                out=flat[:], in0=flat[:], scalar1=HW, scalar2=None, op0=OP.mult
            )
            nc.vector.tensor_tensor(out=flat[:], in0=flat[:], in1=idx, op=OP.add)

            # gather g[m,k] = tags.flat[flat[m,k]]  (510 4-byte descriptors)
            g = sb.tile([MP, K], F32)
            nc.vector.memset(g[:], 0.0)
            nc.gpsimd.indirect_dma_start(
                out=g[:M, :],
                out_offset=None,
                in_=tags[:, :],
                in_offset=bass.IndirectOffsetOnAxis(ap=flat[:], axis=1),
            )

            # visibility mask as f32; padded persons are all-invisible
            mask = sb.tile([MP, K], F32)
            nc.vector.memset(mask[:], 0.0)
            nc.vector.tensor_scalar(
                out=mask[:M, :], in0=vis, scalar1=0, scalar2=None, op0=OP.is_gt
            )

            # cnt -> inv = 1/max(cnt,1)
            cnt = sb.tile([MP, 1], F32)
            nc.vector.tensor_reduce(out=cnt[:], in_=mask[:], axis=AX.X, op=OP.add)
            inv = sb.tile([MP, 1], F32)
            nc.vector.tensor_scalar(
                out=inv[:], in0=cnt[:], scalar1=1.0, scalar2=None, op0=OP.max
            )
            nc.vector.reciprocal(out=inv[:], in_=inv[:])

            # per-person reference tag: mean = sum(g*mask) * inv (0 on pad rows)
            gm = sb.tile([MP, K], F32)
            nc.vector.tensor_tensor(out=gm[:M], in0=g[:M], in1=mask[:M], op=OP.mult)
            mean = sb.tile([MP, 1], F32)
            nc.vector.memset(mean[:], 0.0)
            nc.vector.tensor_reduce(out=mean[:M], in_=gm[:M], axis=AX.X, op=OP.add)
            nc.vector.tensor_tensor(out=mean[:M], in0=mean[:M], in1=inv[:M], op=OP.mult)

            # pull per person: sum(mask*(g-mean)^2) * inv
            dev = sb.tile([MP, K], F32)
            nc.vector.tensor_scalar(
                out=dev[:M],
                in0=g[:M],
                scalar1=mean[:M, 0:1],
                scalar2=None,
                op0=OP.subtract,
            )
            nc.vector.tensor_tensor(out=dev[:M], in0=dev[:M], in1=dev[:M], op=OP.mult)
            nc.vector.tensor_tensor(out=dev[:M], in0=dev[:M], in1=mask[:M], op=OP.mult)
            pull_pp = sb.tile([MP, 1], F32)
            nc.vector.memset(pull_pp[:], 0.0)
            nc.vector.tensor_reduce(out=pull_pp[:M], in_=dev[:M], axis=AX.X, op=OP.add)
            nc.vector.tensor_tensor(
                out=pull_pp[:M], in0=pull_pp[:M], in1=inv[:M], op=OP.mult
            )

            # person validity (0 on pad rows since cnt=0 there)
            pvalid = sb.tile([MP, 1], F32)
            nc.vector.tensor_scalar(
                out=pvalid[:], in0=cnt[:], scalar1=0.0, scalar2=None, op0=OP.is_gt
            )

            # transpose mean and pvalid into row vectors (32x32 DVE transpose)
            meanB = sb.tile([MP, MP], F32)
            nc.vector.tensor_copy(out=meanB[:], in_=mean[:, 0:1].to_broadcast([MP, MP]))
            meanT = sb.tile([MP, MP], F32)
            nc.vector.transpose(out=meanT[:], in_=meanB[:])
            pvB = sb.tile([MP, MP], F32)
            nc.vector.tensor_copy(out=pvB[:], in_=pvalid[:, 0:1].to_broadcast([MP, MP]))
            pvT = sb.tile([MP, MP], F32)
            nc.vector.transpose(out=pvT[:], in_=pvB[:])

            # pair matrix: exp(-(mean_m - mean_n)^2), gated by pvalid_m*pvalid_n
            neg_mean = sb.tile([MP, 1], F32)
            nc.vector.tensor_scalar(
                out=neg_mean[:], in0=mean[:], scalar1=-1.0, scalar2=None, op0=OP.mult
            )
            d2 = sb.tile([MP, MP], F32)
            nc.scalar.activation(
                out=d2[:], in_=meanT[:], func=ACT.Square, bias=neg_mean[:, 0:1], scale=1.0
            )
            e = sb.tile([MP, MP], F32)
            nc.scalar.activation(out=e[:], in_=d2[:], func=ACT.Exp, bias=0.0, scale=-1.0)
            pvpair = sb.tile([MP, MP], F32)
            nc.vector.tensor_scalar(
                out=pvpair[:], in0=pvT[:], scalar1=pvalid[:, 0:1], scalar2=None, op0=OP.mult
            )
            nc.vector.tensor_tensor(out=pvpair[:], in0=pvpair[:], in1=e[:], op=OP.mult)
            push_r = sb.tile([MP, 1], F32)
            nc.vector.tensor_reduce(out=push_r[:], in_=pvpair[:], axis=AX.X, op=OP.add)

            # partition-reduce [pull_pp, push_r, pvalid] with ones^T @ stacked
            stacked = sb.tile([MP, 3], F32)
            nc.vector.tensor_copy(out=stacked[:, 0:1], in_=pull_pp[:])
            nc.vector.tensor_copy(out=stacked[:, 1:2], in_=push_r[:])
            nc.vector.tensor_copy(out=stacked[:, 2:3], in_=pvalid[:])
            ones = sb.tile([MP, 1], F32)
            nc.vector.memset(ones[:], 1.0)
            S_ps = ps.tile([1, 3], F32)
            nc.tensor.matmul(out=S_ps[:], lhsT=ones[:], rhs=stacked[:], start=True, stop=True)
            S = sb.tile([1, 3], F32)
            nc.vector.tensor_copy(out=S[:], in_=S_ps[:])

            # epilogue on partition 0:
            #   pull_loss = pull_sum / max(n,1)
            #   push_loss = (n>1) * push_sum / max(n*(n-1),1) * 0.5
            n_ap = S[0:1, 2:3]
            res = sb.tile([1, 8], F32)
            nc.vector.memset(res[:], 0.0)
            t = sb.tile([1, 4], F32)
            nc.vector.tensor_scalar(
                out=t[0:1, 0:1], in0=n_ap, scalar1=1.0, scalar2=None, op0=OP.max
            )
            nc.vector.reciprocal(out=t[0:1, 0:1], in_=t[0:1, 0:1])
            nc.vector.tensor_tensor(
                out=res[0:1, 0:1], in0=S[0:1, 0:1], in1=t[0:1, 0:1], op=OP.mult
            )
            nc.vector.tensor_tensor(out=t[0:1, 1:2], in0=n_ap, in1=n_ap, op=OP.mult)
            nc.vector.tensor_tensor(
                out=t[0:1, 1:2], in0=t[0:1, 1:2], in1=n_ap, op=OP.subtract
            )
            nc.vector.tensor_scalar(
                out=t[0:1, 1:2], in0=t[0:1, 1:2], scalar1=1.0, scalar2=None, op0=OP.max
            )
            nc.vector.reciprocal(out=t[0:1, 1:2], in_=t[0:1, 1:2])
            nc.vector.tensor_scalar(
                out=t[0:1, 2:3], in0=n_ap, scalar1=1.0, scalar2=None, op0=OP.is_gt
            )
            nc.vector.tensor_tensor(
                out=t[0:1, 3:4], in0=S[0:1, 1:2], in1=t[0:1, 1:2], op=OP.mult
            )
            nc.vector.tensor_scalar(
                out=t[0:1, 3:4], in0=t[0:1, 3:4], scalar1=0.5, scalar2=None, op0=OP.mult
            )
            nc.vector.tensor_tensor(
                out=res[0:1, 1:2], in0=t[0:1, 3:4], in1=t[0:1, 2:3], op=OP.mult
            )

            # AllReduce the two scalars across the 8 cores (32B padded buffers)
            cc_in = dram.tile([1, 8], F32)
            cc_out = dram.tile([1, 8], F32)
            nc.gpsimd.dma_start(out=cc_in[:], in_=res[:])
            nc.gpsimd.collective_compute(
                "AllReduce",
                OP.add,
                replica_groups=[list(range(NCORES))],
                ins=[cc_in.opt()],
                outs=[cc_out.opt()],
            )
            nc.gpsimd.dma_start(out=out[:, :], in_=cc_out[0:1, 0:2])

    if finalize:
        nc.finalize()
    return nc


_NC_CACHE = None


def _get_nc():
    global _NC_CACHE
    if _NC_CACHE is None:
        _NC_CACHE = build_nc()
    return _NC_CACHE


def make_in_maps(tags, keypoint_indices):
    tags = np.ascontiguousarray(np.asarray(tags, dtype=np.float32))
    kp = np.ascontiguousarray(np.asarray(keypoint_indices, dtype=np.int32))
    assert tags.shape == (B, K, HW), tags.shape
    assert kp.shape == (B, M, K, 2), kp.shape
    return [{"tags": tags[i], "kp": kp[i]} for i in range(NCORES)]


def kernel(tags, keypoint_indices, **run_kwargs):
    nc = _get_nc()
    in_maps = make_in_maps(tags, keypoint_indices)
    r = run_bass_kernel_spmd(nc, in_maps, core_ids=list(range(NCORES)), **run_kwargs)
    out = np.asarray(r.results[0]["out"], dtype=np.float32)
    pull = np.asarray(out[0, 0], dtype=np.float32)
    push = np.asarray(out[0, 1], dtype=np.float32)
    return (pull, push)


# revision 10
# speedup vs baseline: 3.7759x; 3.7759x over previous
"""Associative-embedding (AE) loss kernel for Trainium2, 8 NeuronCores.

Strategy (data-parallel over batch, per the sharding hint):
  - B=8 images, 8 cores -> one image per core.
  - Per core: gather the 30x17 tag values tags[k, idx[m,k]] with indirect
    (gather) DMAs -- only 510 x 4B of the 17MB tag map is touched.
    The HW DGE consumes ONE index per output partition row, so the 510
    indices are laid out as [102, 5] and gathered with 5 calls of [102, 1].
  - Tiny DVE/ACT/PE compute produces the per-image (pull, push) scalars.
  - Each core writes its per-image partial; the batch sum (the unshard of a
    data-parallel loss) happens on the host over the 8 pairs.
    (An ncfw AllReduce of the two scalars was measured at 70-150us on this
    stack -- 5x the rest of the kernel -- so it is deliberately avoided.)
"""

import numpy as np

import concourse.bass as bass
import concourse.bacc as bacc
import concourse.tile as tile
from concourse import mybir
from concourse.bass_utils import run_bass_kernel_spmd

B, K, HW, M = 8, 17, 262144, 30
NCORES = 8
MP = 32  # person dim padded to the DVE stream-transpose block size
GP, GC = 128, 4  # gather layout: 510 indices padded to 128 partitions x 4 cols

F32 = mybir.dt.float32
I32 = mybir.dt.int32
AX = mybir.AxisListType
OP = mybir.AluOpType
ACT = mybir.ActivationFunctionType


def build_nc(finalize=True):
    nc = bacc.Bacc(None, num_devices=NCORES)
    tags = nc.declare_dram_parameter("tags", [K, HW], F32, isOutput=False)
    kp = nc.declare_dram_parameter("kp", [M, K, 2], I32, isOutput=False)
    out = nc.declare_dram_parameter("out", [1, 2], F32, isOutput=True)

    with tile.TileContext(nc) as tc:
        with (
            tc.tile_pool(name="sb", bufs=1) as sb,
            tc.tile_pool(name="ps", bufs=1, space="PSUM") as ps,
            tc.tile_pool(name="dram", bufs=1, space="DRAM") as dram,
        ):
            # keypoints -> SBUF; partition = person m
            kp_t = sb.tile([M, K, 2], I32)
            nc.sync.dma_start(out=kp_t[:], in_=kp[:, :, :])
            idx = kp_t[:, :, 0]
            vis = kp_t[:, :, 1]

            # flat gather index: idx[m,k] + k*HW (iota step is int16-limited,
            # so emit k then scale by HW on the DVE)
            flat = sb.tile([M, K], I32)
            nc.gpsimd.iota(flat[:], pattern=[[1, K]], base=0, channel_multiplier=0)
            nc.vector.tensor_scalar(
                out=flat[:], in0=flat[:], scalar1=HW, scalar2=None, op0=OP.mult
            )
            nc.vector.tensor_tensor(out=flat[:], in0=flat[:], in1=idx, op=OP.add)

            # The HW DGE consumes ONE index per output partition row, so lay
            # the 510 indices out as [128, 4] (padded with 2 dupes of index 0)
            # and gather with 4 calls of [128, 1]. The cross-partition
            # rearrange bounces through contiguous DRAM, where reshape views
            # are free (the DMA engine cannot restream mismatched runs).
            iscr = dram.tile([1, GP * GC], I32)
            v_i_mk = iscr[0:1, 0 : M * K].rearrange("o (m k) -> (o m) k", k=K)
            v_i_pc = iscr[:, :].rearrange("o (p c) -> (o p) c", c=GC)
            nc.sync.dma_start(out=v_i_mk, in_=flat[:, :])
            nc.sync.dma_start(out=iscr[0:1, M * K :], in_=flat[0:1, 0 : GP * GC - M * K])
            icols = sb.tile([GP, GC], I32)
            nc.sync.dma_start(out=icols[:, :], in_=v_i_pc)

            gcols = sb.tile([GP, GC], F32)
            for c in range(GC):
                nc.gpsimd.indirect_dma_start(
                    out=gcols[:, c : c + 1],
                    out_offset=None,
                    in_=tags[:, :],
                    in_offset=bass.IndirectOffsetOnAxis(
                        ap=icols[:, c : c + 1], axis=1
                    ),
                )

            # back to [30, 17] person-major layout (again via DRAM bounce)
            gscr = dram.tile([1, GP * GC], F32)
            v_g_mk = gscr[0:1, 0 : M * K].rearrange("o (m k) -> (o m) k", k=K)
            v_g_pc = gscr[:, :].rearrange("o (p c) -> (o p) c", c=GC)
            nc.sync.dma_start(out=v_g_pc, in_=gcols[:, :])
            g = sb.tile([MP, K], F32)
            nc.vector.memset(g[:], 0.0)
            nc.sync.dma_start(out=g[:M, :], in_=v_g_mk)

            # visibility mask as f32; padded persons are all-invisible
            mask = sb.tile([MP, K], F32)
            nc.vector.memset(mask[:], 0.0)
            nc.vector.tensor_scalar(
                out=mask[:M, :], in0=vis, scalar1=0, scalar2=None, op0=OP.is_gt
            )

            # cnt -> inv = 1/max(cnt,1)
            cnt = sb.tile([MP, 1], F32)
            nc.vector.tensor_reduce(out=cnt[:], in_=mask[:], axis=AX.X, op=OP.add)
            inv = sb.tile([MP, 1], F32)
            nc.vector.tensor_scalar(
                out=inv[:], in0=cnt[:], scalar1=1.0, scalar2=None, op0=OP.max
            )
            nc.vector.reciprocal(out=inv[:], in_=inv[:])

            # per-person reference tag: mean = sum(g*mask) * inv (0 on pad rows)
            gm = sb.tile([MP, K], F32)
            nc.vector.tensor_tensor(out=gm[:M], in0=g[:M], in1=mask[:M], op=OP.mult)
            mean = sb.tile([MP, 1], F32)
            nc.vector.memset(mean[:], 0.0)
            nc.vector.tensor_reduce(out=mean[:M], in_=gm[:M], axis=AX.X, op=OP.add)
            nc.vector.tensor_tensor(out=mean[:M], in0=mean[:M], in1=inv[:M], op=OP.mult)

            # pull per person: sum(mask*(g-mean)^2) * inv
            dev = sb.tile([MP, K], F32)
            nc.vector.tensor_scalar(
                out=dev[:M],
                in0=g[:M],
                scalar1=mean[:M, 0:1],
                scalar2=None,
                op0=OP.subtract,
            )
            nc.vector.tensor_tensor(out=dev[:M], in0=dev[:M], in1=dev[:M], op=OP.mult)
            nc.vector.tensor_tensor(out=dev[:M], in0=dev[:M], in1=mask[:M], op=OP.mult)
            pull_pp = sb.tile([MP, 1], F32)
            nc.vector.memset(pull_pp[:], 0.0)
            nc.vector.tensor_reduce(out=pull_pp[:M], in_=dev[:M], axis=AX.X, op=OP.add)
            nc.vector.tensor_tensor(
                out=pull_pp[:M], in0=pull_pp[:M], in1=inv[:M], op=OP.mult
            )

            # person validity (0 on pad rows since cnt=0 there)
            pvalid = sb.tile([MP, 1], F32)
            nc.vector.tensor_scalar(
                out=pvalid[:], in0=cnt[:], scalar1=0.0, scalar2=None, op0=OP.is_gt
            )

            # transpose mean and pvalid into row vectors (32x32 DVE transpose)
            meanB = sb.tile([MP, MP], F32)
            nc.vector.tensor_copy(out=meanB[:], in_=mean[:, 0:1].to_broadcast([MP, MP]))
            meanT = sb.tile([MP, MP], F32)
            nc.vector.transpose(out=meanT[:], in_=meanB[:])
            pvB = sb.tile([MP, MP], F32)
            nc.vector.tensor_copy(out=pvB[:], in_=pvalid[:, 0:1].to_broadcast([MP, MP]))
            pvT = sb.tile([MP, MP], F32)
            nc.vector.transpose(out=pvT[:], in_=pvB[:])

            # pair matrix: exp(-(mean_m - mean_n)^2), gated by pvalid_m*pvalid_n
            neg_mean = sb.tile([MP, 1], F32)
            nc.vector.tensor_scalar(
                out=neg_mean[:], in0=mean[:], scalar1=-1.0, scalar2=None, op0=OP.mult
            )
            d2 = sb.tile([MP, MP], F32)
            nc.scalar.activation(
                out=d2[:], in_=meanT[:], func=ACT.Square, bias=neg_mean[:, 0:1], scale=1.0
            )
            e = sb.tile([MP, MP], F32)
            nc.scalar.activation(out=e[:], in_=d2[:], func=ACT.Exp, bias=0.0, scale=-1.0)
            pvpair = sb.tile([MP, MP], F32)
            nc.vector.tensor_scalar(
                out=pvpair[:], in0=pvT[:], scalar1=pvalid[:, 0:1], scalar2=None, op0=OP.mult
            )
            nc.vector.tensor_tensor(out=pvpair[:], in0=pvpair[:], in1=e[:], op=OP.mult)
            push_r = sb.tile([MP, 1], F32)
            nc.vector.tensor_reduce(out=push_r[:], in_=pvpair[:], axis=AX.X, op=OP.add)

            # partition-reduce [pull_pp, push_r, pvalid] with ones^T @ stacked
            stacked = sb.tile([MP, 3], F32)
            nc.vector.tensor_copy(out=stacked[:, 0:1], in_=pull_pp[:])
            nc.vector.tensor_copy(out=stacked[:, 1:2], in_=push_r[:])
            nc.vector.tensor_copy(out=stacked[:, 2:3], in_=pvalid[:])
            ones = sb.tile([MP, 1], F32)
            nc.vector.memset(ones[:], 1.0)
            S_ps = ps.tile([1, 3], F32)
            nc.tensor.matmul(out=S_ps[:], lhsT=ones[:], rhs=stacked[:], start=True, stop=True)
            S = sb.tile([1, 3], F32)
            nc.vector.tensor_copy(out=S[:], in_=S_ps[:])

            # epilogue on partition 0:
            #   pull_loss = pull_sum / max(n,1)
            #   push_loss = (n>1) * push_sum / max(n*(n-1),1) * 0.5
            n_ap = S[0:1, 2:3]
            res = sb.tile([1, 2], F32)
            t = sb.tile([1, 4], F32)
            nc.vector.tensor_scalar(
                out=t[0:1, 0:1], in0=n_ap, scalar1=1.0, scalar2=None, op0=OP.max
            )
            nc.vector.reciprocal(out=t[0:1, 0:1], in_=t[0:1, 0:1])
            nc.vector.tensor_tensor(
                out=res[0:1, 0:1], in0=S[0:1, 0:1], in1=t[0:1, 0:1], op=OP.mult
            )
            nc.vector.tensor_tensor(out=t[0:1, 1:2], in0=n_ap, in1=n_ap, op=OP.mult)
            nc.vector.tensor_tensor(
                out=t[0:1, 1:2], in0=t[0:1, 1:2], in1=n_ap, op=OP.subtract
            )
            nc.vector.tensor_scalar(
                out=t[0:1, 1:2], in0=t[0:1, 1:2], scalar1=1.0, scalar2=None, op0=OP.max
            )
            nc.vector.reciprocal(out=t[0:1, 1:2], in_=t[0:1, 1:2])
            nc.vector.tensor_scalar(
                out=t[0:1, 2:3], in0=n_ap, scalar1=1.0, scalar2=None, op0=OP.is_gt
            )
            nc.vector.tensor_tensor(
                out=t[0:1, 3:4], in0=S[0:1, 1:2], in1=t[0:1, 1:2], op=OP.mult
            )
            nc.vector.tensor_scalar(
                out=t[0:1, 3:4], in0=t[0:1, 3:4], scalar1=0.5, scalar2=None, op0=OP.mult
            )
            nc.vector.tensor_tensor(
                out=res[0:1, 1:2], in0=t[0:1, 3:4], in1=t[0:1, 2:3], op=OP.mult
            )

            # per-core partial (pull_b, push_b) -> DRAM
            nc.sync.dma_start(out=out[:, :], in_=res[:, :])

    if finalize:
        nc.finalize()
    return nc


_NC_CACHE = None


def _get_nc():
    global _NC_CACHE
    if _NC_CACHE is None:
        _NC_CACHE = build_nc()
    return _NC_CACHE


def make_in_maps(tags, keypoint_indices):
    tags = np.ascontiguousarray(np.asarray(tags, dtype=np.float32))
    kp = np.ascontiguousarray(np.asarray(keypoint_indices, dtype=np.int32))
    assert tags.shape == (B, K, HW), tags.shape
    assert kp.shape == (B, M, K, 2), kp.shape
    return [{"tags": tags[i], "kp": kp[i]} for i in range(NCORES)]


def kernel(tags, keypoint_indices, **run_kwargs):
    nc = _get_nc()
    in_maps = make_in_maps(tags, keypoint_indices)
    r = run_bass_kernel_spmd(nc, in_maps, core_ids=list(range(NCORES)), **run_kwargs)
    parts = np.stack(
        [np.asarray(r.results[i]["out"], dtype=np.float32)[0] for i in range(NCORES)]
    )  # [8, 2] per-image (pull, push)
    pull = np.float32(parts[:, 0].sum(dtype=np.float32))
    push = np.float32(parts[:, 1].sum(dtype=np.float32))
    return (np.asarray(pull), np.asarray(push))


# revision 15
# speedup vs baseline: 4.1720x; 1.1049x over previous
"""Associative-embedding (AE) loss kernel for Trainium2, 8 NeuronCores.

Strategy (data-parallel over batch, per the sharding hint):
  - B=8 images, 8 cores -> one image per core.
  - Per core: gather the 30x17 tag values tags[k, idx[m,k]] with indirect
    (gather) DMAs -- only 510 x 4B of the 17MB tag map is touched.
    The HW DGE consumes ONE index per output partition row, so the k<16
    indices are respread to [120, 4] (SBUF->SBUF DMA, runs 16 -> 4) and
    gathered with 4 calls; the k=16 column is one [30, 1] call using
    element_offset=16*HW directly on the keypoint tile.
  - Tiny DVE/ACT/PE compute produces the per-image (pull, push) scalars.
  - Each core writes its per-image partial; the batch sum (the unshard of a
    data-parallel loss) happens on the host over the 8 pairs.
    (An ncfw AllReduce of the two scalars was measured at 70-150us on this
    stack -- 5x the rest of the kernel -- so it is deliberately avoided.)
"""

import numpy as np

import concourse.bass as bass
import concourse.bacc as bacc
import concourse.tile as tile
from concourse import mybir
from concourse.bass_utils import run_bass_kernel_spmd

B, K, HW, M = 8, 17, 262144, 30
NCORES = 8
MP = 32  # person dim padded to the DVE stream-transpose block size
KL = 16  # k<16 columns gathered via the respread path
GP, GC = 120, 4  # respread layout: 480 = 120 partitions x 4 columns

F32 = mybir.dt.float32
I32 = mybir.dt.int32
AX = mybir.AxisListType
OP = mybir.AluOpType
ACT = mybir.ActivationFunctionType


def build_nc(finalize=True):
    nc = bacc.Bacc(None, num_devices=NCORES)
    tags = nc.declare_dram_parameter("tags", [K, HW], F32, isOutput=False)
    kp = nc.declare_dram_parameter("kp", [M, K, 2], I32, isOutput=False)
    out = nc.declare_dram_parameter("out", [1, 2], F32, isOutput=True)

    with tile.TileContext(nc) as tc:
        with (
            tc.tile_pool(name="sb", bufs=1) as sb,
            tc.tile_pool(name="ps", bufs=1, space="PSUM") as ps,
        ):
            # keypoints -> SBUF; partition = person m
            kp_t = sb.tile([M, K, 2], I32)
            nc.sync.dma_start(out=kp_t[:], in_=kp[:, :, :])
            idx = kp_t[:, :, 0]
            vis = kp_t[:, :, 1]

            # flat gather index: idx[m,k] + k*HW (iota step is int16-limited,
            # so emit k then scale by HW on the DVE)
            flat = sb.tile([M, K], I32)
            nc.gpsimd.iota(flat[:], pattern=[[1, K]], base=0, channel_multiplier=0)
            nc.vector.tensor_scalar(
                out=flat[:], in0=flat[:], scalar1=HW, scalar2=None, op0=OP.mult
            )
            nc.vector.tensor_tensor(out=flat[:], in0=flat[:], in1=idx, op=OP.add)

            # respread [30, :16] -> [120, 4]: one index per partition per call
            # (bounced through contiguous DRAM where reshape views are free)
            iscr = nc.dram_tensor("iscr", [GP * GC], I32)
            nc.sync.dma_start(
                out=iscr[:].rearrange("(m k) -> m k", k=KL), in_=flat[:, 0:KL]
            )
            icols = sb.tile([GP, GC], I32)
            nc.sync.dma_start(
                out=icols[:, :], in_=iscr[:].rearrange("(p c) -> p c", c=GC)
            )

            g = sb.tile([MP, K], F32)
            nc.vector.memset(g[:], 0.0)

            # k=16 column is already one-index-per-partition: gather directly
            nc.gpsimd.indirect_dma_start(
                out=g[:M, KL:K],
                out_offset=None,
                in_=tags[:, :],
                in_offset=bass.IndirectOffsetOnAxis(ap=flat[:, KL:K], axis=1),
            )

            gcols = sb.tile([GP, GC], F32)
            for c in range(GC):
                nc.gpsimd.indirect_dma_start(
                    out=gcols[:, c : c + 1],
                    out_offset=None,
                    in_=tags[:, :],
                    in_offset=bass.IndirectOffsetOnAxis(
                        ap=icols[:, c : c + 1], axis=1
                    ),
                )
            nc.sync.dma_start(out=g[:M, 0:KL], in_=gcols[:, :])

            # visibility mask as f32; padded persons are all-invisible
            mask = sb.tile([MP, K], F32)
            nc.vector.memset(mask[:], 0.0)
            nc.vector.tensor_scalar(
                out=mask[:M, :], in0=vis, scalar1=0, scalar2=None, op0=OP.is_gt
            )

            # cnt -> inv = 1/max(cnt,1)
            cnt = sb.tile([MP, 1], F32)
            nc.vector.tensor_reduce(out=cnt[:], in_=mask[:], axis=AX.X, op=OP.add)
            inv = sb.tile([MP, 1], F32)
            nc.vector.tensor_scalar(
                out=inv[:], in0=cnt[:], scalar1=1.0, scalar2=None, op0=OP.max
            )
            nc.vector.reciprocal(out=inv[:], in_=inv[:])

            # stacked holds the three per-person columns to partition-reduce:
            # [pull_pp, push_row, pvalid]
            stacked = sb.tile([MP, 3], F32)
            nc.vector.memset(stacked[:, 0:1], 0.0)

            # person validity (0 on pad rows since cnt=0 there)
            nc.vector.tensor_scalar(
                out=stacked[:, 2:3], in0=cnt[:], scalar1=0.0, scalar2=None, op0=OP.is_gt
            )
            pvalid = stacked[:, 2:3]

            # per-person reference tag: mean = sum(g*mask) * inv (0 on pad rows)
            gm = sb.tile([MP, K], F32)
            mean = sb.tile([MP, 1], F32)
            nc.vector.memset(mean[:], 0.0)
            nc.vector.tensor_tensor(out=gm[:M], in0=g[:M], in1=mask[:M], op=OP.mult)
            nc.vector.tensor_reduce(out=mean[:M], in_=gm[:M], axis=AX.X, op=OP.add)
            nc.vector.tensor_tensor(out=mean[:M], in0=mean[:M], in1=inv[:M], op=OP.mult)

            # pull per person: sum(mask*(g-mean)^2) * inv  (mask^2 == mask)
            dev = sb.tile([MP, K], F32)
            nc.vector.tensor_scalar(
                out=dev[:M],
                in0=g[:M],
                scalar1=mean[:M, 0:1],
                scalar2=None,
                op0=OP.subtract,
            )
            nc.vector.tensor_tensor(out=dev[:M], in0=dev[:M], in1=mask[:M], op=OP.mult)
            dsq = sb.tile([MP, K], F32)
            spull = sb.tile([MP, 1], F32)
            nc.vector.tensor_tensor(out=dsq[:M], in0=dev[:M], in1=dev[:M], op=OP.mult)
            nc.vector.tensor_reduce(out=spull[:M], in_=dsq[:M], axis=AX.X, op=OP.add)
            nc.vector.tensor_tensor(
                out=stacked[:M, 0:1], in0=spull[:M], in1=inv[:M], op=OP.mult
            )

            # broadcast mean|pvalid along free dim, then one blocked 32x32
            # transpose turns both into row vectors
            mpB = sb.tile([MP, 2 * MP], F32)
            nc.vector.tensor_copy(
                out=mpB[:, 0:MP], in_=mean[:, 0:1].to_broadcast([MP, MP])
            )
            nc.vector.tensor_copy(
                out=mpB[:, MP : 2 * MP], in_=pvalid.to_broadcast([MP, MP])
            )
            mpT = sb.tile([MP, 2 * MP], F32)
            nc.vector.transpose(out=mpT[:], in_=mpB[:])
            meanT = mpT[:, 0:MP]
            pvT = mpT[:, MP : 2 * MP]

            # pair matrix: exp(-(mean_m - mean_n)^2), gated by pvalid_m*pvalid_n
            neg_mean = sb.tile([MP, 1], F32)
            nc.vector.tensor_scalar(
                out=neg_mean[:], in0=mean[:], scalar1=-1.0, scalar2=None, op0=OP.mult
            )
            d2 = sb.tile([MP, MP], F32)
            nc.scalar.activation(
                out=d2[:], in_=meanT, func=ACT.Square, bias=neg_mean[:, 0:1], scale=1.0
            )
            e = sb.tile([MP, MP], F32)
            nc.scalar.activation(out=e[:], in_=d2[:], func=ACT.Exp, bias=0.0, scale=-1.0)
            pvpair = sb.tile([MP, MP], F32)
            nc.vector.tensor_scalar(
                out=pvpair[:], in0=pvT, scalar1=pvalid, scalar2=None, op0=OP.mult
            )
            pe_s = sb.tile([MP, MP], F32)
            nc.vector.tensor_tensor(out=pe_s[:], in0=pvpair[:], in1=e[:], op=OP.mult)
            nc.vector.tensor_reduce(out=stacked[:, 1:2], in_=pe_s[:], axis=AX.X, op=OP.add)

            # partition-reduce the three columns with ones^T @ stacked
            ones = sb.tile([MP, 1], F32)
            nc.vector.memset(ones[:], 1.0)
            S_ps = ps.tile([1, 3], F32)
            nc.tensor.matmul(
                out=S_ps[:], lhsT=ones[:], rhs=stacked[:], start=True, stop=True
            )
            S = sb.tile([1, 3], F32)
            nc.vector.tensor_copy(out=S[:], in_=S_ps[:])

            # epilogue on partition 0:
            #   pull_loss = pull_sum / max(n,1)
            #   push_loss = (n>1) * push_sum / max(n*(n-1),1) * 0.5
            n_ap = S[0:1, 2:3]
            res = sb.tile([1, 2], F32)
            t = sb.tile([1, 4], F32)
            nc.vector.tensor_scalar(
                out=t[0:1, 0:1], in0=n_ap, scalar1=1.0, scalar2=None, op0=OP.max
            )
            nc.vector.reciprocal(out=t[0:1, 0:1], in_=t[0:1, 0:1])
            nc.vector.tensor_tensor(
                out=res[0:1, 0:1], in0=S[0:1, 0:1], in1=t[0:1, 0:1], op=OP.mult
            )
            nc.vector.tensor_tensor(out=t[0:1, 1:2], in0=n_ap, in1=n_ap, op=OP.mult)
            nc.vector.tensor_tensor(
                out=t[0:1, 1:2], in0=t[0:1, 1:2], in1=n_ap, op=OP.subtract
            )
            nc.vector.tensor_scalar(
                out=t[0:1, 1:2], in0=t[0:1, 1:2], scalar1=1.0, scalar2=None, op0=OP.max
            )
            nc.vector.reciprocal(out=t[0:1, 1:2], in_=t[0:1, 1:2])
            nc.vector.tensor_tensor(
                out=t[0:1, 2:3], in0=S[0:1, 1:2], in1=t[0:1, 1:2], op=OP.mult
            )
            nc.vector.tensor_scalar(
                out=t[0:1, 3:4],
                in0=n_ap,
                scalar1=1.0,
                scalar2=0.5,
                op0=OP.is_gt,
                op1=OP.mult,
            )
            nc.vector.tensor_tensor(
                out=res[0:1, 1:2], in0=t[0:1, 2:3], in1=t[0:1, 3:4], op=OP.mult
            )

            # per-core partial (pull_b, push_b) -> DRAM
            nc.sync.dma_start(out=out[:, :], in_=res[:, :])

    if finalize:
        nc.finalize()
    return nc


_NC_CACHE = None


def _get_nc():
    global _NC_CACHE
    if _NC_CACHE is None:
        _NC_CACHE = build_nc()
    return _NC_CACHE


def make_in_maps(tags, keypoint_indices):
    tags = np.ascontiguousarray(np.asarray(tags, dtype=np.float32))
    kp = np.ascontiguousarray(np.asarray(keypoint_indices, dtype=np.int32))
    assert tags.shape == (B, K, HW), tags.shape
    assert kp.shape == (B, M, K, 2), kp.shape
    return [{"tags": tags[i], "kp": kp[i]} for i in range(NCORES)]


def kernel(tags, keypoint_indices, **run_kwargs):
    nc = _get_nc()
    in_maps = make_in_maps(tags, keypoint_indices)
    r = run_bass_kernel_spmd(nc, in_maps, core_ids=list(range(NCORES)), **run_kwargs)
    parts = np.stack(
        [np.asarray(r.results[i]["out"], dtype=np.float32)[0] for i in range(NCORES)]
    )  # [8, 2] per-image (pull, push)
    pull = np.float32(parts[:, 0].sum(dtype=np.float32))
    push = np.float32(parts[:, 1].sum(dtype=np.float32))
    return (np.asarray(pull), np.asarray(push))


# revision 17
# speedup vs baseline: 5.2921x; 1.2685x over previous
"""Associative-embedding (AE) loss kernel for Trainium2, 8 NeuronCores.

Strategy (data-parallel over batch, per the sharding hint):
  - B=8 images, 8 cores -> one image per core.
  - Per core: gather the 30x17 tag values tags[k, idx[m,k]] with indirect
    (gather) DMAs -- only 510 x 4B of the 17MB tag map is touched.
    The HW DGE consumes ONE index per output partition row, so the k<16
    indices are respread to [120, 4] (SBUF->SBUF DMA, runs 16 -> 4) and
    gathered with 4 calls; the k=16 column is one [30, 1] call using
    element_offset=16*HW directly on the keypoint tile.
  - Tiny DVE/ACT/PE compute produces the per-image (pull, push) scalars.
  - Each core writes its per-image partial; the batch sum (the unshard of a
    data-parallel loss) happens on the host over the 8 pairs.
    (An ncfw AllReduce of the two scalars was measured at 70-150us on this
    stack -- 5x the rest of the kernel -- so it is deliberately avoided.)
"""

import numpy as np

import concourse.bass as bass
import concourse.bacc as bacc
import concourse.tile as tile
from concourse import mybir
from concourse.bass_utils import run_bass_kernel_spmd

B, K, HW, M = 8, 17, 262144, 30
NCORES = 8
MP = 32  # person dim padded to the DVE stream-transpose block size
KL = 16  # k<16 columns gathered via the respread path
GP, GC = 120, 4  # respread layout: 480 = 120 partitions x 4 columns

F32 = mybir.dt.float32
I32 = mybir.dt.int32
AX = mybir.AxisListType
OP = mybir.AluOpType
ACT = mybir.ActivationFunctionType


def build_nc(finalize=True):
    nc = bacc.Bacc(None, num_devices=NCORES)
    tags = nc.declare_dram_parameter("tags", [K, HW], F32, isOutput=False)
    kp = nc.declare_dram_parameter("kp", [M, K, 2], I32, isOutput=False)
    out = nc.declare_dram_parameter("out", [1, 2], F32, isOutput=True)

    with tile.TileContext(nc) as tc:
        with (
            tc.tile_pool(name="sb", bufs=1) as sb,
            tc.tile_pool(name="ps", bufs=1, space="PSUM") as ps,
        ):
            # keypoints -> SBUF; partition = person m
            kp_t = sb.tile([M, K, 2], I32)
            nc.sync.dma_start(out=kp_t[:], in_=kp[:, :, :])
            idx = kp_t[:, :, 0]
            vis = kp_t[:, :, 1]

            # k<16 indices loaded straight from DRAM into the [120, 4] gather
            # layout (the HW DGE consumes one index per partition row), then
            # the k*HW row offset is added in-layout: k = (4p + c) & 15
            icols = sb.tile([GP, GC], I32)
            nc.sync.dma_start(out=icols[:, :], in_=kp[:, 0:KL, 0])
            kofs = sb.tile([GP, GC], I32)
            nc.gpsimd.iota(kofs[:], pattern=[[1, GC]], base=0, channel_multiplier=GC)
            nc.vector.tensor_scalar(
                out=kofs[:], in0=kofs[:], scalar1=KL - 1, scalar2=None, op0=OP.bitwise_and
            )
            nc.vector.tensor_scalar(
                out=kofs[:], in0=kofs[:], scalar1=HW, scalar2=None, op0=OP.mult
            )
            nc.vector.tensor_tensor(out=icols[:], in0=icols[:], in1=kofs[:], op=OP.add)

            g = sb.tile([MP, K], F32)
            nc.vector.memset(g[:], 0.0)

            # k=16 column is already one-index-per-partition: gather directly
            flat16 = sb.tile([M, 1], I32)
            nc.vector.tensor_scalar(
                out=flat16[:],
                in0=kp_t[:, KL, 0:1],
                scalar1=KL * HW,
                scalar2=None,
                op0=OP.add,
            )
            nc.gpsimd.indirect_dma_start(
                out=g[:M, KL:K],
                out_offset=None,
                in_=tags[:, :],
                in_offset=bass.IndirectOffsetOnAxis(ap=flat16[:], axis=1),
            )

            gcols = sb.tile([GP, GC], F32)
            for c in range(GC):
                nc.gpsimd.indirect_dma_start(
                    out=gcols[:, c : c + 1],
                    out_offset=None,
                    in_=tags[:, :],
                    in_offset=bass.IndirectOffsetOnAxis(
                        ap=icols[:, c : c + 1], axis=1
                    ),
                )
            nc.sync.dma_start(out=g[:M, 0:KL], in_=gcols[:, :])

            # visibility mask as f32; padded persons are all-invisible
            mask = sb.tile([MP, K], F32)
            nc.vector.memset(mask[:], 0.0)
            nc.vector.tensor_scalar(
                out=mask[:M, :], in0=vis, scalar1=0, scalar2=None, op0=OP.is_gt
            )

            # cnt -> inv = 1/max(cnt,1)
            cnt = sb.tile([MP, 1], F32)
            nc.vector.tensor_reduce(out=cnt[:], in_=mask[:], axis=AX.X, op=OP.add)
            inv = sb.tile([MP, 1], F32)
            nc.vector.tensor_scalar(
                out=inv[:], in0=cnt[:], scalar1=1.0, scalar2=None, op0=OP.max
            )
            nc.vector.reciprocal(out=inv[:], in_=inv[:])

            # stacked holds the three per-person columns to partition-reduce:
            # [pull_pp, push_row, pvalid]
            stacked = sb.tile([MP, 3], F32)
            nc.vector.memset(stacked[:, 0:1], 0.0)

            # person validity (0 on pad rows since cnt=0 there)
            nc.vector.tensor_scalar(
                out=stacked[:, 2:3], in0=cnt[:], scalar1=0.0, scalar2=None, op0=OP.is_gt
            )
            pvalid = stacked[:, 2:3]

            # per-person reference tag: mean = sum(g*mask) * inv (0 on pad rows)
            gm = sb.tile([MP, K], F32)
            mean = sb.tile([MP, 1], F32)
            nc.vector.memset(mean[:], 0.0)
            nc.vector.tensor_tensor(out=gm[:M], in0=g[:M], in1=mask[:M], op=OP.mult)
            nc.vector.tensor_reduce(out=mean[:M], in_=gm[:M], axis=AX.X, op=OP.add)
            nc.vector.tensor_tensor(out=mean[:M], in0=mean[:M], in1=inv[:M], op=OP.mult)

            # pull per person: sum(mask*(g-mean)^2) * inv  (mask^2 == mask)
            dev = sb.tile([MP, K], F32)
            nc.vector.tensor_scalar(
                out=dev[:M],
                in0=g[:M],
                scalar1=mean[:M, 0:1],
                scalar2=None,
                op0=OP.subtract,
            )
            nc.vector.tensor_tensor(out=dev[:M], in0=dev[:M], in1=mask[:M], op=OP.mult)
            dsq = sb.tile([MP, K], F32)
            spull = sb.tile([MP, 1], F32)
            nc.vector.tensor_tensor(out=dsq[:M], in0=dev[:M], in1=dev[:M], op=OP.mult)
            nc.vector.tensor_reduce(out=spull[:M], in_=dsq[:M], axis=AX.X, op=OP.add)
            nc.vector.tensor_tensor(
                out=stacked[:M, 0:1], in0=spull[:M], in1=inv[:M], op=OP.mult
            )

            # broadcast mean|pvalid along free dim, then one blocked 32x32
            # transpose turns both into row vectors
            mpB = sb.tile([MP, 2 * MP], F32)
            nc.vector.tensor_copy(
                out=mpB[:, 0:MP], in_=mean[:, 0:1].to_broadcast([MP, MP])
            )
            nc.vector.tensor_copy(
                out=mpB[:, MP : 2 * MP], in_=pvalid.to_broadcast([MP, MP])
            )
            mpT = sb.tile([MP, 2 * MP], F32)
            nc.vector.transpose(out=mpT[:], in_=mpB[:])
            meanT = mpT[:, 0:MP]
            pvT = mpT[:, MP : 2 * MP]

            # pair matrix: exp(-(mean_m - mean_n)^2), gated by pvalid_m*pvalid_n
            neg_mean = sb.tile([MP, 1], F32)
            nc.vector.tensor_scalar(
                out=neg_mean[:], in0=mean[:], scalar1=-1.0, scalar2=None, op0=OP.mult
            )
            d2 = sb.tile([MP, MP], F32)
            nc.scalar.activation(
                out=d2[:], in_=meanT, func=ACT.Square, bias=neg_mean[:, 0:1], scale=1.0
            )
            e = sb.tile([MP, MP], F32)
            nc.scalar.activation(out=e[:], in_=d2[:], func=ACT.Exp, bias=0.0, scale=-1.0)
            pvpair = sb.tile([MP, MP], F32)
            nc.vector.tensor_scalar(
                out=pvpair[:], in0=pvT, scalar1=pvalid, scalar2=None, op0=OP.mult
            )
            pe_s = sb.tile([MP, MP], F32)
            nc.vector.tensor_tensor(out=pe_s[:], in0=pvpair[:], in1=e[:], op=OP.mult)
            nc.vector.tensor_reduce(out=stacked[:, 1:2], in_=pe_s[:], axis=AX.X, op=OP.add)

            # partition-reduce the three columns with ones^T @ stacked
            ones = sb.tile([MP, 1], F32)
            nc.vector.memset(ones[:], 1.0)
            S_ps = ps.tile([1, 3], F32)
            nc.tensor.matmul(
                out=S_ps[:], lhsT=ones[:], rhs=stacked[:], start=True, stop=True
            )
            S = sb.tile([1, 3], F32)
            nc.vector.tensor_copy(out=S[:], in_=S_ps[:])

            # epilogue on partition 0:
            #   pull_loss = pull_sum / max(n,1)
            #   push_loss = (n>1) * push_sum / max(n*(n-1),1) * 0.5
            n_ap = S[0:1, 2:3]
            res = sb.tile([1, 2], F32)
            t = sb.tile([1, 4], F32)
            nc.vector.tensor_scalar(
                out=t[0:1, 0:1], in0=n_ap, scalar1=1.0, scalar2=None, op0=OP.max
            )
            nc.vector.reciprocal(out=t[0:1, 0:1], in_=t[0:1, 0:1])
            nc.vector.tensor_tensor(
                out=res[0:1, 0:1], in0=S[0:1, 0:1], in1=t[0:1, 0:1], op=OP.mult
            )
            nc.vector.tensor_tensor(out=t[0:1, 1:2], in0=n_ap, in1=n_ap, op=OP.mult)
            nc.vector.tensor_tensor(
                out=t[0:1, 1:2], in0=t[0:1, 1:2], in1=n_ap, op=OP.subtract
            )
            nc.vector.tensor_scalar(
                out=t[0:1, 1:2], in0=t[0:1, 1:2], scalar1=1.0, scalar2=None, op0=OP.max
            )
            nc.vector.reciprocal(out=t[0:1, 1:2], in_=t[0:1, 1:2])
            nc.vector.tensor_tensor(
                out=t[0:1, 2:3], in0=S[0:1, 1:2], in1=t[0:1, 1:2], op=OP.mult
            )
            nc.vector.tensor_scalar(
                out=t[0:1, 3:4],
                in0=n_ap,
                scalar1=1.0,
                scalar2=0.5,
                op0=OP.is_gt,
                op1=OP.mult,
            )
            nc.vector.tensor_tensor(
                out=res[0:1, 1:2], in0=t[0:1, 2:3], in1=t[0:1, 3:4], op=OP.mult
            )

            # per-core partial (pull_b, push_b) -> DRAM
            nc.sync.dma_start(out=out[:, :], in_=res[:, :])

    if finalize:
        nc.finalize()
    return nc


_NC_CACHE = None


def _get_nc():
    global _NC_CACHE
    if _NC_CACHE is None:
        _NC_CACHE = build_nc()
    return _NC_CACHE


def make_in_maps(tags, keypoint_indices):
    tags = np.ascontiguousarray(np.asarray(tags, dtype=np.float32))
    kp = np.ascontiguousarray(np.asarray(keypoint_indices, dtype=np.int32))
    assert tags.shape == (B, K, HW), tags.shape
    assert kp.shape == (B, M, K, 2), kp.shape
    return [{"tags": tags[i], "kp": kp[i]} for i in range(NCORES)]


def kernel(tags, keypoint_indices, **run_kwargs):
    nc = _get_nc()
    in_maps = make_in_maps(tags, keypoint_indices)
    r = run_bass_kernel_spmd(nc, in_maps, core_ids=list(range(NCORES)), **run_kwargs)
    parts = np.stack(
        [np.asarray(r.results[i]["out"], dtype=np.float32)[0] for i in range(NCORES)]
    )  # [8, 2] per-image (pull, push)
    pull = np.float32(parts[:, 0].sum(dtype=np.float32))
    push = np.float32(parts[:, 1].sum(dtype=np.float32))
    return (np.asarray(pull), np.asarray(push))
